# revision 20
# baseline (speedup 1.0000x reference)
"""3-layer GAT message passing on 8 Trainium2 NeuronCores (Bass/Tile).

v2: chunk-batched phase B with uniform 5A+5B subtiles per 64-dst window.

Sharding: nodes split into 8 contiguous dst-ranges with ~equal edge counts.
Each core's nodes pack into 64-node windows (total degree cap keeps every
window to <=5 A-subtiles and <=5 B-subtiles). Windows renumber nodes into a
window-padded layout: node at (core c, window g, offset o) -> table row
c*OWNP + g*64 + o. The per-layer node table is split into two halves by
LOCAL row (r < OWNP/2 -> half A), each half AllGather'ed separately so
A-region gathers start after the first collective. Gather indices are
int16-safe (max 8*OWNP/2).

Per layer:
  phase A (own rows, batched 4 tiles/chunk): h_t = h @ Wcat -> fp16 table
  rows [h(128)|asrc|pad] (layer 2: [h(64)|1|asrc|pad], 256B rows); adst
  stays in SBUF (adstS column per tile).
  2x AllGather (half tables) -> full fp16 tables in each core's HBM.
  phase B per chunk (8 windows, 80 subtiles uniform): 4 dma_gathers on
  SWDGE queues 0-3 (desc-gen runs on disjoint Q7 core pairs, ~4x parallel),
  then chunk-wide DVE ops: S one-hot build, adst one-hot dot, z/lrelu/exp,
  Sw, denominator tree; 80 psOT matmuls accumulate the transposed output
  [feat, 512 nodes] in one PSUM bank; ONE hrow (relu+bias) feeds the next
  layer's phase A directly from SBUF.
Layer 2 keeps node-major psO with in-row ones (denominator), fuses
global_add_pool into a persistent PSUM bank and a final AllReduce.
"""

import dataclasses
import math

import numpy as np


@dataclasses.dataclass
class Cfg:
    n_nodes: int = 50000
    n_edges: int = 800000
    in_c: int = 128
    hid_c: int = 128
    out_c: int = 64
    n_graphs: int = 64
    neg_slope: float = 0.2
    ncores: int = 8
    win: int = 64            # dst window (<=64 nodes per group)
    nsub: int = 5            # uniform subtiles per region per window
    gpc: int = 8             # windows per chunk
    degcap: int = 1000       # total-degree cap per window (keeps A/B <= 640)


FULL = Cfg()


# ----------------------------------------------------------------------------
# host-side planning (pure numpy)
# ----------------------------------------------------------------------------

def build_plan(edge_index, batch, cfg: Cfg):
    degcap = cfg.degcap
    while True:
        try:
            return _build_plan(edge_index, batch, cfg, degcap)
        except AssertionError:
            degcap -= 60
            assert degcap >= 600, "window packing failed"


def _build_plan(edge_index, batch, cfg: Cfg, degcap):
    N = cfg.n_nodes
    src0 = np.concatenate([edge_index[0].astype(np.int64), np.arange(N)])
    dst0 = np.concatenate([edge_index[1].astype(np.int64), np.arange(N)])
    order = np.argsort(dst0, kind="stable")
    src_g = src0[order]
    dst_g = dst0[order]
    E = src_g.shape[0]

    deg = np.bincount(dst_g, minlength=N)
    # balance cores on expected window count: a window closes at 64 nodes
    # or degcap edges, so cost(node) = max(1/win, deg/degcap)
    wcost = np.maximum(1.0 / cfg.win, deg / degcap)
    cumw = np.cumsum(wcost)
    bounds = [0]
    for c in range(1, cfg.ncores):
        bounds.append(int(np.searchsorted(cumw, c * cumw[-1] / cfg.ncores)))
    bounds.append(N)

    # windows: <=64 nodes and total degree <= degcap
    core_windows = []
    for c in range(cfg.ncores):
        lo_n, hi_n = bounds[c], bounds[c + 1]
        wins = []
        w0 = lo_n
        while w0 < hi_n:
            w1, tot = w0, 0
            while (w1 < hi_n and (w1 - w0) < cfg.win
                   and tot + deg[w1] <= degcap):
                tot += deg[w1]
                w1 += 1
            if w1 == w0:
                w1 = w0 + 1
            wins.append(np.arange(w0, w1))
            w0 = w1
        # snake-order by node count so the two local table halves hold
        # ~equal node totals (balances the A/B gather split)
        order_w = np.argsort([-len(w) for w in wins], kind="stable")
        snake = []
        li, ri = 0, len(wins) - 1
        for i, oi in enumerate(order_w):
            if i % 2 == 0:
                snake.append((li, oi)); li += 1
            else:
                snake.append((ri, oi)); ri -= 1
        placed = [None] * len(wins)
        for pos, oi in snake:
            placed[pos] = wins[oi]
        core_windows.append(placed)

    gmax = max(len(w) for w in core_windows)
    gtot = math.ceil(gmax / cfg.gpc) * cfg.gpc
    nchunk = gtot // cfg.gpc
    ownp = gtot * cfg.win
    # half-table split at a chunk boundary (may be asymmetric)
    halfa = (nchunk // 2) * cfg.gpc * cfg.win
    halfb = ownp - halfa
    assert ownp % 512 == 0
    assert cfg.ncores * max(halfa, halfb) <= 32768, f"idx overflow: {ownp}"

    # remap: original node -> (core, local row)
    remap = np.full(N, -1, np.int64)
    for c in range(cfg.ncores):
        for g, nodes in enumerate(core_windows[c]):
            remap[nodes] = c * ownp + g * cfg.win + np.arange(len(nodes))
    assert (remap[np.unique(src_g)] >= 0).all()

    # half-table gather index per source node
    lrow = remap % ownp
    score = remap // ownp
    in_a = lrow < halfa
    gidx = np.where(in_a, score * halfa + lrow,
                    score * halfb + (lrow - halfa))

    NSUB, W, GPC = cfg.nsub, cfg.win, cfg.gpc
    spw = NSUB * 128          # slots per region per window
    spcr = GPC * spw          # slots per region per chunk (5120)

    plans = []
    for c in range(cfg.ncores):
        lo = np.searchsorted(dst_g, bounds[c])
        hi = np.searchsorted(dst_g, bounds[c + 1])
        s_all = src_g[lo:hi]
        d_all = dst_g[lo:hi]
        nstart = np.searchsorted(d_all, np.arange(bounds[c], bounds[c + 1] + 1))

        idxA = np.zeros(nchunk * spcr, np.int64)
        idxB = np.zeros(nchunk * spcr, np.int64)
        dstl = np.full((128, nchunk * 2 * GPC * NSUB), -1.0, np.float16)
        batchloc = np.full((W, gtot), -1.0, np.float16)
        for g, nodes in enumerate(core_windows[c]):
            k, gi = divmod(g, GPC)
            e0 = nstart[nodes[0] - bounds[c]]
            e1 = nstart[nodes[-1] + 1 - bounds[c]]
            s = s_all[e0:e1]
            dloc = (d_all[e0:e1] - nodes[0]).astype(np.int64)
            a_mask = in_a[s]
            for (reg, m) in ((0, a_mask), (1, ~a_mask)):
                sv = gidx[s[m]]
                dv = dloc[m]
                n = len(sv)
                assert n <= spw, f"window overflow: core {c} g {g} reg {reg} n {n}"
                it = idxA if reg == 0 else idxB
                s0 = k * spcr + gi * spw
                it[s0:s0 + n] = sv
                # dsl column for slot i (region-major per chunk):
                # col = k*80 + reg*40 + gi*5 + i//128
                col0 = k * 2 * GPC * NSUB + reg * GPC * NSUB + gi * NSUB
                ii = np.arange(n)
                dstl[ii % 128, col0 + ii // 128] = dv
            batchloc[:len(nodes), g] = batch[nodes].astype(np.float16)

        # wrap indices into the dma_gather int16 layout (per gather call:
        # half-chunk granularity 2560 idx so each of the 4 queue calls is
        # contiguous)
        def wrap(arr):
            # arr: [nchunk*spcr]; per chunk contiguous; wrap 16-partition
            t = arr.reshape(-1, 16)            # [n/16, 16]
            out = np.zeros((128, len(arr) // 16), np.int16)
            out[:16, :] = t.T.astype(np.int16)
            out[:, :] = np.tile(out[:16, :], (8, 1))
            return out

        # per-chunk wrapping (idx layout is [128, num_idxs//16] per call;
        # calls cover 20-subtile halves: 2560 idx each)
        idxAw = np.zeros((128, nchunk * spcr // 16), np.int16)
        idxBw = np.zeros((128, nchunk * spcr // 16), np.int16)
        for k in range(nchunk):
            for h_i in range(2):
                o0 = k * spcr + h_i * (spcr // 2)
                seg = np.arange(o0, o0 + spcr // 2)
                for (arr, outw) in ((idxA, idxAw), (idxB, idxBw)):
                    t = np.zeros((16, spcr // 2 // 16), np.int16)
                    a = arr[seg]
                    t[np.arange(len(a)) % 16, np.arange(len(a)) // 16] = \
                        a.astype(np.int16)
                    outw[:, o0 // 16:(o0 + spcr // 2) // 16] = np.tile(t, (8, 1))

        # blob: per chunk [ixa 640B | ixb 640B | dsl 160B] = 1440B per row
        bpc = spcr // 16 * 2 * 2 + 2 * GPC * NSUB * 2
        blob = np.zeros((128, nchunk * bpc), np.uint8)
        for k in range(nchunk):
            o = k * bpc
            blob[:, o:o + 640] = idxAw[:, k * 320:(k + 1) * 320].view(np.uint8)
            blob[:, o + 640:o + 1280] = \
                idxBw[:, k * 320:(k + 1) * 320].view(np.uint8)
            blob[:, o + 1280:o + 1440] = \
                np.ascontiguousarray(dstl[:, k * 80:(k + 1) * 80]).view(np.uint8)
        plans.append({"blob": blob, "batchloc": batchloc,
                      "windows": core_windows[c]})

    meta = {"gtot": gtot, "nchunk": nchunk, "ownp": ownp, "halfa": halfa,
            "halfb": halfb, "bpc": bpc}
    return plans, meta


# ----------------------------------------------------------------------------
# device program (shared across all 8 cores)
# ----------------------------------------------------------------------------

def build_nc(cfg: Cfg, meta, debug=False):
    import concourse.bass as bass  # noqa: F401
    import concourse.mybir as mybir
    import concourse.tile as tile
    from concourse import bacc

    fp16 = mybir.dt.float16
    f32 = mybir.dt.float32
    i16 = mybir.dt.int16
    u8 = mybir.dt.uint8
    AL = mybir.AluOpType
    AF = mybir.ActivationFunctionType
    AX = mybir.AxisListType

    gtot, nchunk = meta["gtot"], meta["nchunk"]
    OWNP = meta["ownp"]
    HALFA, HALFB = meta["halfa"], meta["halfb"]
    BPC = meta["bpc"]
    W, GPC, NSUB = cfg.win, cfg.gpc, cfg.nsub
    NREG = GPC * NSUB          # 40 subtiles per region per chunk
    SPC = 2 * NREG             # 80 subtiles per chunk
    couts = [cfg.hid_c, cfg.hid_c, cfg.out_c]
    cins = [cfg.in_c, cfg.hid_c, cfg.hid_c]
    rowb = [512, 512, 256]     # table row bytes per layer
    rg = [list(range(cfg.ncores))]
    throws = [cfg.ncores * HALFA, cfg.ncores * HALFB]
    ntiles = OWNP // 128

    nc = bacc.Bacc("TRN2", target_bir_lowering=False, debug=debug,
                   num_swdge_queues=4)

    xT = nc.dram_tensor("xT", [cfg.in_c, OWNP], fp16, kind="ExternalInput")
    Wc = []
    wcols = [couts[0] + 2, couts[1] + 2, couts[2] + 2]
    for l in range(3):
        Wc.append(nc.dram_tensor(f"wcat{l}", [cins[l], wcols[l]], fp16,
                                 kind="ExternalInput"))
    biasT = [nc.dram_tensor(f"biasT{l}", [couts[l], 1], f32,
                            kind="ExternalInput") for l in range(2)]
    bias2 = nc.dram_tensor("bias2", [W, cfg.out_c], f32, kind="ExternalInput")
    iota64_d = nc.dram_tensor("iota64", [128, W], fp16, kind="ExternalInput")
    onesr_d = nc.dram_tensor("onesrow", [1, 128], fp16, kind="ExternalInput")
    onescol_d = nc.dram_tensor("onescol", [128, 1], fp16, kind="ExternalInput")
    ones128_d = nc.dram_tensor("ones128", [128, 128], fp16, kind="ExternalInput")
    mask128_d = nc.dram_tensor("mask128", [128, 512], fp16, kind="ExternalInput")
    blob_d = nc.dram_tensor("blob", [128, nchunk * BPC], u8, kind="ExternalInput")
    bloc_d = nc.dram_tensor("batchloc", [W, gtot], fp16, kind="ExternalInput")
    out_ext = nc.dram_tensor("out", [cfg.n_graphs, cfg.out_c], f32,
                             kind="ExternalOutput")

    ownt, tblH = [], []
    for l in range(3):
        rc = rowb[l] // 2
        ownt.append(nc.dram_tensor(f"ownt{l}", [OWNP, rc], fp16))
        tblH.append([nc.dram_tensor(f"tbl{l}h{h}", [throws[h], rc], fp16,
                                    addr_space="Shared") for h in range(2)])
    pool_l = nc.dram_tensor("pool_local", [cfg.n_graphs, cfg.out_c], f32)
    pool_s = nc.dram_tensor("pool_shared", [cfg.n_graphs, cfg.out_c], f32,
                            addr_space="Shared")

    import contextlib
    with tile.TileContext(nc) as tc, contextlib.ExitStack() as ctx:
        cpool = ctx.enter_context(tc.tile_pool(name="consts", bufs=1))
        apool = ctx.enter_context(tc.tile_pool(name="phasea", bufs=2))
        wpool = ctx.enter_context(tc.tile_pool(name="work", bufs=2))
        wpool1 = ctx.enter_context(tc.tile_pool(name="work1", bufs=1))
        psa = ctx.enter_context(tc.tile_pool(name="psa", bufs=1, space="PSUM"))
        pspool = ctx.enter_context(tc.tile_pool(name="ps", bufs=1, space="PSUM"))
        psot1 = ctx.enter_context(tc.tile_pool(name="psot1", bufs=1, space="PSUM"))
        psacc = ctx.enter_context(tc.tile_pool(name="psacc", bufs=1, space="PSUM"))

        from concourse import library_config
        nc.gpsimd.load_library(library_config.mlp)

        # ---- constants ----
        iota64 = cpool.tile([128, W], fp16, tag="iota")
        nc.sync.dma_start(iota64[:], iota64_d[:, :])
        onesr = cpool.tile([1, 128], fp16, tag="onesr")
        nc.sync.dma_start(onesr[:], onesr_d[:, :])
        onescol = cpool.tile([128, 1], fp16, tag="onescol")
        nc.sync.dma_start(onescol[:], onescol_d[:, :])
        ones128 = cpool.tile([128, 128], fp16, tag="ones128")
        nc.sync.dma_start(ones128[:], ones128_d[:, :])
        mask128 = cpool.tile([128, 512], fp16, tag="mask128")
        nc.sync.dma_start(mask128[:], mask128_d[:, :])
        onesr32 = cpool.tile([1, 128], f32, tag="onesr32")
        nc.vector.tensor_copy(out=onesr32[:], in_=onesr[:])
        wcat_sb = []
        for l in range(3):
            t = cpool.tile([cins[l], wcols[l]], fp16, tag=f"wc{l}")
            nc.sync.dma_start(t[:], Wc[l][:, :])
            wcat_sb.append(t)
        biasT_sb = []
        for l in range(2):
            t = cpool.tile([couts[l], 1], f32, tag=f"bT{l}")
            nc.sync.dma_start(t[:], biasT[l][:, :])
            biasT_sb.append(t)
        bias2_sb = cpool.tile([W, cfg.out_c], f32, tag="b2")
        nc.sync.dma_start(bias2_sb[:], bias2[:, :])
        bloc_sb = cpool.tile([W, gtot], fp16, tag="bloc")
        nc.sync.dma_start(bloc_sb[:], bloc_d[:, :])
        # blob tiles: per-chunk rotating loads (layer-invariant indices)
        bpool = ctx.enter_context(tc.tile_pool(name="blob", bufs=3))

        # persistent gather buffers (3-deep rotation for 2-chunk prefetch)
        gbufA = [cpool.tile([128, NREG, 512], u8, tag=f"gA{p}", name=f"gA{p}")
                 for p in (0, 1, 2)]
        gbufB = [cpool.tile([128, NREG, 512], u8, tag=f"gB{p}", name=f"gB{p}")
                 for p in (0, 1, 2)]
        for p in (0, 1, 2):
            nc.vector.memset(gbufA[p][:].bitcast(fp16), 0.0)
            nc.vector.memset(gbufB[p][:].bitcast(fp16), 0.0)

        def g2view(buf):
            # layer-2 256B-row view of the first half of a 512B-row buffer
            return buf[:, 0:NREG // 2, :].rearrange(
                "p s (a e) -> p (s a) e", a=2)

        # adst columns per tile, per layer
        adstS = [cpool.tile([128, ntiles], fp16, tag=f"adstS{l}",
                            name=f"adstS{l}") for l in range(3)]
        # layer-2 table packing buffer: col 64 preset to 1.0
        tblt2 = cpool.tile([128, 4, 128], fp16, tag="tblt2")
        nc.vector.memset(tblt2[:], 0.0)
        nc.vector.memset(tblt2[:, :, 64:65], 1.0)

        psum_pool_acc = psacc.tile([cfg.n_graphs, cfg.out_c], f32, tag="pool")

        blob_tiles = {}

        def emit_blob_load(k):
            bt = bpool.tile([128, BPC], u8, tag="blob")
            nc.sync.dma_start(bt[:], blob_d[:, k * BPC:(k + 1) * BPC])
            blob_tiles[k] = bt

        def blob_views(k):
            bt = blob_tiles[k]
            ixa = bt[:, 0:640].bitcast(i16)
            ixb = bt[:, 640:1280].bitcast(i16)
            dsl = bt[:, 1280:1440].bitcast(fp16)
            return ixa, ixb, dsl

        def emit_phase_a(l, k, lhsT_full):
            """4 tiles (512 own nodes) of layer-l table; lhsT_full is the
            [cin, 512] fp16 SBUF tile holding h_prev for these nodes."""
            if l < 2:
                tblt = apool.tile([128, 4, 256], fp16, tag="tblt")
            for t in range(4):
                psA = psa.tile([128, wcols[l]], f32, tag="pa", space="PSUM")
                nc.tensor.matmul(out=psA[:],
                                 lhsT=lhsT_full[:, t * 128:(t + 1) * 128],
                                 rhs=wcat_sb[l][:], start=True, stop=True)
                if l < 2:
                    nc.vector.tensor_copy(out=tblt[:, t, 0:129],
                                          in_=psA[:, 0:129])
                    nc.vector.memset(tblt[:, t, 129:256], 0.0)
                else:
                    nc.vector.tensor_copy(out=tblt2[:, t, 0:64],
                                          in_=psA[:, 0:64])
                    nc.vector.tensor_copy(out=tblt2[:, t, 65:66],
                                          in_=psA[:, 64:65])
                nc.vector.tensor_copy(out=adstS[l][:, 4 * k + t:4 * k + t + 1],
                                      in_=psA[:, wcols[l] - 1:wcols[l]])
            src = tblt if l < 2 else tblt2
            nc.scalar.dma_start(
                ownt[l][k * 512:(k + 1) * 512, :].rearrange(
                    "(t p) c -> p t c", p=128),
                src[:])

        def emit_gathers(l, k):
            ixa, ixb, dsl = blob_views(k)
            HN = 2560  # idx per gather call
            if l < 2:
                gA, gB = gbufA[k % 3][:], gbufB[k % 3][:]
                eb = 512
            else:
                gA, gB = g2view(gbufA[k % 3]), g2view(gbufB[k % 3])
                eb = 256
            tA = tblH[l][0][:, :].bitcast(u8)
            tB = tblH[l][1][:, :].bitcast(u8)
            nc.gpsimd.dma_gather(gA[:, 0:20, :], tA, ixa[:, 0:160],
                                 HN, HN, eb, single_packet=False, queue_num=0)
            nc.gpsimd.dma_gather(gA[:, 20:40, :], tA, ixa[:, 160:320],
                                 HN, HN, eb, single_packet=False, queue_num=1)
            nc.gpsimd.dma_gather(gB[:, 0:20, :], tB, ixb[:, 0:160],
                                 HN, HN, eb, single_packet=False, queue_num=2)
            nc.gpsimd.dma_gather(gB[:, 20:40, :], tB, ixb[:, 160:320],
                                 HN, HN, eb, single_packet=False, queue_num=3)

        front_state = {}

        def emit_front(l, k):
            """chunk pre-work independent of the gathered data."""
            _, _, dsl = blob_views(k)
            # awbc: broadcast adst of the chunk's 512 own nodes:
            # awbc[p, 128*t + p'] = adstS[p', 4k+t]
            raw = wpool.tile([128, 4, 128], fp16, tag="awraw")
            nc.vector.tensor_tensor(
                out=raw[:],
                in0=adstS[l][:, 4 * k:4 * k + 4][:, :, None]
                .to_broadcast([128, 4, 128]),
                in1=mask128[:].rearrange("p (t q) -> p t q", t=4),
                op=AL.mult)
            psaw = pspool.tile([128, 512], f32, tag="psaw", space="PSUM")
            nc.tensor.matmul(out=psaw[:],
                             lhsT=ones128[:],
                             rhs=raw[:].rearrange("p t q -> p (t q)"),
                             start=True, stop=True)
            awbc = wpool.tile([128, GPC, W], fp16, tag="awbc")
            nc.vector.tensor_copy(out=awbc[:],
                                  in_=psaw[:].rearrange("p (g w) -> p g w",
                                                        g=GPC))
            S = wpool.tile([128, 2, GPC, NSUB, W], fp16, tag="S")
            Sf = S[:].rearrange("p r g j w -> p (r g j) w")
            nc.vector.tensor_tensor(
                out=Sf,
                in0=iota64[:][:, None, :].to_broadcast([128, SPC, W]),
                in1=dsl[:, :, None].to_broadcast([128, SPC, W]),
                op=AL.is_equal)
            st = {"awbc": awbc, "S": S}
            if l == 2:
                bselc = wpool.tile([W, GPC, cfg.n_graphs], fp16, tag="bselc")
                nc.vector.tensor_tensor(
                    out=bselc[:],
                    in0=iota64[0:W, 0:cfg.n_graphs][:, None, :]
                    .to_broadcast([W, GPC, cfg.n_graphs]),
                    in1=bloc_sb[:, k * GPC:(k + 1) * GPC][:, :, None]
                    .to_broadcast([W, GPC, cfg.n_graphs]),
                    op=AL.is_equal)
                st["bselc"] = bselc
            front_state[(l, k)] = st

        def emit_weights(l, k, gAh, gBh, ac):
            """the gather-dependent edge-weight pipeline; returns Sw."""
            st = front_state.pop((l, k))
            awbc, S = st["awbc"], st["S"]
            Sf = S[:].rearrange("p r g j w -> p (r g j) w")

            asrcT = wpool.tile([128, SPC], f32, tag="asrcT")
            nc.scalar.activation(
                out=asrcT[:, 0:NREG],
                in_=gAh[:, :, ac:ac + 1].rearrange("p s x -> p (s x)"),
                func=AF.Copy)
            nc.scalar.activation(
                out=asrcT[:, NREG:SPC],
                in_=gBh[:, :, ac:ac + 1].rearrange("p s x -> p (s x)"),
                func=AF.Copy)

            # adstT = one-hot dot of S with awbc
            adstT = wpool.tile([128, SPC], fp16, tag="adstT")
            tmpa = wpool1.tile([128, GPC, NSUB, W], fp16, tag="tmpa")
            for r in range(2):
                nc.vector.tensor_tensor(
                    out=tmpa[:], in0=S[:, r],
                    in1=awbc[:][:, :, None, :]
                    .to_broadcast([128, GPC, NSUB, W]),
                    op=AL.mult)
                with nc.allow_low_precision("one-hot select"):
                    nc.vector.tensor_reduce(
                        out=adstT[:, r * NREG:(r + 1) * NREG],
                        in_=tmpa[:].rearrange("p g j w -> p (g j) w"),
                        axis=AX.X, op=AL.add)

            z = wpool.tile([128, SPC], f32, tag="z")
            nc.vector.tensor_add(out=z[:], in0=asrcT[:], in1=adstT[:])
            zl = wpool.tile([128, SPC], f32, tag="zl")
            nc.vector.scalar_tensor_tensor(
                out=zl[:], in0=z[:], scalar=cfg.neg_slope, in1=z[:],
                op0=AL.mult, op1=AL.max)
            wv = wpool.tile([128, SPC], fp16, tag="wv")
            nc.scalar.activation(out=wv[:], in_=zl[:], func=AF.Exp)

            Sw = wpool1.tile([128, 2, GPC, NSUB, W], fp16, tag="Sw")
            nc.vector.tensor_tensor(
                out=Sw[:].rearrange("p r g j w -> p (r g j) w"),
                in0=Sf,
                in1=wv[:][:, :, None].to_broadcast([128, SPC, W]),
                op=AL.mult)
            return st, Sw

        def emit_back_01(l, k):
            """gather-dependent part for layers 0/1; returns hrow [128,512]."""
            gAh = gbufA[k % 3][:].bitcast(fp16)   # [128, 40, 256]
            gBh = gbufB[k % 3][:].bitcast(fp16)
            st, Sw = emit_weights(l, k, gAh, gBh, 128)

            # transposed scatter + PE denominator per window
            psOT = psot1.tile([128, 512], f32, tag="psOT", space="PSUM")
            psD = pspool.tile([1, 512], f32, tag="psD", space="PSUM")
            for gi in range(GPC):
                for j in range(2 * NSUB):
                    if j < NSUB:
                        gh = gAh[:, NSUB * gi + j, 0:128]
                        sw = Sw[:, 0, gi, j, :]
                    else:
                        gh = gBh[:, NSUB * gi + (j - NSUB), 0:128]
                        sw = Sw[:, 1, gi, j - NSUB, :]
                    nc.tensor.matmul(out=psOT[:, gi * W:(gi + 1) * W],
                                     lhsT=gh, rhs=sw,
                                     start=(j == 0), stop=(j == 2 * NSUB - 1))
                    nc.tensor.matmul(out=psD[:, gi * W:(gi + 1) * W],
                                     lhsT=onescol[:], rhs=sw,
                                     start=(j == 0), stop=(j == 2 * NSUB - 1))
            psDe = wpool.tile([1, 512], f32, tag="psDe")
            nc.vector.tensor_scalar_add(psDe[:], psD[:], 1e-30)
            psR = pspool.tile([128, 512], f32, tag="psR", space="PSUM")
            nc.tensor.matmul(out=psR[:], lhsT=onesr32[:], rhs=psDe[:],
                             start=True, stop=True)
            recbc = wpool.tile([128, 512], f32, tag="recbc")
            nc.vector.reciprocal(out=recbc[:], in_=psR[:])
            tdiv = wpool.tile([128, 512], fp16, tag="tdiv")
            nc.vector.tensor_tensor(out=tdiv[:], in0=psOT[:], in1=recbc[:],
                                    op=AL.mult)
            hrow = wpool.tile([128, 512], fp16, tag="hrow")
            nc.scalar.activation(out=hrow[:], in_=tdiv[:], func=AF.Relu,
                                 bias=biasT_sb[l][:], scale=1.0)
            return hrow

        def emit_back_2(l, k):
            """gather-dependent part for layer 2 + pool fusion."""
            gAh = g2view(gbufA[k % 3]).bitcast(fp16)    # [128, 40, 128]
            gBh = g2view(gbufB[k % 3]).bitcast(fp16)
            st, Sw = emit_weights(l, k, gAh, gBh, 65)
            bselc = st["bselc"]

            psO = [psot1.tile([W, 4, 65], f32, tag=f"psO{h}", space="PSUM",
                              name=f"psO{h}")
                   for h in range(2)]
            for gi in range(GPC):
                po = psO[gi // 4][:, gi % 4, :]
                for j in range(2 * NSUB):
                    if j < NSUB:
                        gh = gAh[:, NSUB * gi + j, 0:65]
                        sw = Sw[:, 0, gi, j, :]
                    else:
                        gh = gBh[:, NSUB * gi + (j - NSUB), 0:65]
                        sw = Sw[:, 1, gi, j - NSUB, :]
                    nc.tensor.matmul(out=po, lhsT=sw, rhs=gh,
                                     start=(j == 0), stop=(j == 2 * NSUB - 1))
            den = wpool.tile([W, GPC], f32, tag="den")
            for h in range(2):
                nc.vector.tensor_scalar_add(
                    den[:, h * 4:(h + 1) * 4],
                    psO[h][:, :, 64:65].rearrange("p a x -> p (a x)"), 1e-30)
            rec = wpool.tile([W, GPC], f32, tag="rec")
            nc.vector.reciprocal(out=rec[:], in_=den[:])
            for gi in range(GPC):
                g = k * GPC + gi
                oh = wpool.tile([W, cfg.out_c], fp16, tag="oh")
                nc.vector.scalar_tensor_tensor(
                    out=oh[:], in0=psO[gi // 4][:, gi % 4, 0:64],
                    scalar=rec[:, gi:gi + 1], in1=bias2_sb[:],
                    op0=AL.mult, op1=AL.add)
                nc.tensor.matmul(out=psum_pool_acc[:],
                                 lhsT=bselc[:, gi, :], rhs=oh[:],
                                 start=(g == 0), stop=(g == gtot - 1))

        # ================= main program =================
        NH = nchunk // 2

        def emit_ag(l, h):
            lo = 0 if h == 0 else HALFA
            hi = HALFA if h == 0 else OWNP
            nc.gpsimd.collective_compute(
                "AllGather", AL.bypass, replica_groups=rg,
                ins=[ownt[l][lo:hi, :]], outs=[tblH[l][h][:, :]])

        # phase A layer 0 (batched; xT chunks loaded from DRAM); the first
        # AllGather half triggers as soon as its rows are stored
        for k in range(min(3, nchunk)):
            emit_blob_load(k)
        for k in range(nchunk):
            lhsT = apool.tile([cfg.in_c, 512], fp16, tag="lhsT0")
            nc.scalar.dma_start(lhsT[:], xT[:, k * 512:(k + 1) * 512])
            emit_phase_a(0, k, lhsT)
            if k == NH - 1:
                emit_ag(0, 0)
        emit_ag(0, 1)

        for l in range(3):
            emit_gathers(l, 0)
            emit_gathers(l, 1)
            emit_front(l, 0)
            for k in range(nchunk):
                if k + 3 < nchunk:
                    emit_blob_load(k + 3)
                if k + 2 < nchunk:
                    emit_gathers(l, k + 2)
                if k + 1 < nchunk:
                    emit_front(l, k + 1)
                if l < 2:
                    hrow = emit_back_01(l, k)
                    emit_phase_a(l + 1, k, hrow)
                    if k == NH - 1:
                        emit_ag(l + 1, 0)
                    if k == nchunk - 1:
                        emit_ag(l + 1, 1)
                else:
                    emit_back_2(l, k)
            if l < 2:
                # blob tiles for the next layer (rotation restarts at 0)
                for k in range(min(3, nchunk)):
                    emit_blob_load(k)

        # ---- pool -> allreduce -> out ----
        pooled = cpool.tile([cfg.n_graphs, cfg.out_c], f32, tag="pooled")
        nc.vector.tensor_copy(out=pooled[:], in_=psum_pool_acc[:])
        nc.sync.dma_start(pool_l[:, :], pooled[:])
        nc.gpsimd.collective_compute(
            "AllReduce", AL.add, replica_groups=rg,
            ins=[pool_l[:, :]], outs=[pool_s[:, :]])
        nc.sync.dma_start(out_ext[:, :], pool_s[:, :])

    nc.compile()
    return nc


# ----------------------------------------------------------------------------
# host wrapper
# ----------------------------------------------------------------------------

def make_inputs(inputs, plans, meta, cfg: Cfg):
    x = np.asarray(inputs["x"], np.float32)
    ownp, gtot = meta["ownp"], meta["gtot"]
    iota64 = np.tile(np.arange(cfg.win, dtype=np.float16), (128, 1))
    onesrow = np.ones((1, 128), np.float16)
    onescol = np.ones((128, 1), np.float16)
    ones128 = np.ones((128, 128), np.float16)
    mask128 = np.zeros((128, 512), np.float16)
    for t in range(4):
        mask128[np.arange(128), t * 128 + np.arange(128)] = 1.0
    wcats = []
    for l in range(3):
        Wl = np.asarray(inputs[f"W{l}"], np.float32)
        asl = np.asarray(inputs[f"as{l}"], np.float32)
        adl = np.asarray(inputs[f"ad{l}"], np.float32)
        wc = np.concatenate([Wl, (Wl @ asl)[:, None], (Wl @ adl)[:, None]],
                            axis=1)
        wcats.append(wc.astype(np.float16))
    biasTs = [np.asarray(inputs[f"b{l}"], np.float32)[:, None] for l in range(2)]
    bias2 = np.tile(np.asarray(inputs["b2"], np.float32)[None, :], (cfg.win, 1))
    in_maps = []
    for c in range(cfg.ncores):
        p = plans[c]
        xpad = np.zeros((ownp, cfg.in_c), np.float32)
        for g, nodes in enumerate(p["windows"]):
            xpad[g * cfg.win: g * cfg.win + len(nodes)] = x[nodes]
        m = {"xT": np.ascontiguousarray(xpad.T).astype(np.float16),
             "iota64": iota64, "onesrow": onesrow, "onescol": onescol,
             "ones128": ones128, "mask128": mask128,
             "blob": p["blob"], "batchloc": p["batchloc"],
             "biasT0": biasTs[0], "biasT1": biasTs[1], "bias2": bias2}
        for l in range(3):
            m[f"wcat{l}"] = wcats[l]
        in_maps.append(m)
    return in_maps


def kernel(**inputs) -> np.ndarray:
    cfg = FULL
    edge_index = np.asarray(inputs["edge_index"])
    batch = np.asarray(inputs["batch"])
    plans, meta = build_plan(edge_index, batch, cfg)
    in_maps = make_inputs(inputs, plans, meta, cfg)
    nc = build_nc(cfg, meta, debug=False)
    from concourse import bass_utils
    res = bass_utils.run_bass_kernel_spmd(nc, in_maps, core_ids=list(range(cfg.ncores)))
    return np.asarray(res.results[0]["out"], np.float32)
